# revision 11
# baseline (speedup 1.0000x reference)
"""CrossScan3D Trainium2 kernel.

Computes, for input x[B=2, C=96, 32, 32, 32] f32, the stack of 12 scans
out[B, 12, C, L=32768]: the 6 axis-order flattenings {ijk, ikj, jki, jik,
kij, kji} of each (b, c) 32^3 volume plus their reversals, in the channel
order of the reference:

    s=0: ijk   s=1: ikj   s=2: rev-ijk   s=3: rev-ikj
    s=4: jki   s=5: jik   s=6: rev-jki   s=7: rev-jik
    s=8: kij   s=9: kji   s=10: rev-kij  s=11: rev-kji

Pure data movement; rel-err tolerance is 2e-2, so the DRAM output is bf16
(max rounding rel err ~3.9e-3) and upcast to f32 on the host — this halves
the HBM write traffic that is the roofline. Sharding: the 192 (b, c)
volumes split 24 per core across 8 cores (no communication).

Per core, volumes are processed 8 at a time in [128, 2048] bf16 SBUF
tiles: partition p = v*32 + a (v in 0..3), free = u*1024 + f (u in 0..1),
with volume = base + 4u + v. The load DMA (SWDGE/gpsimd ring) casts
f32->bf16 in flight. Per supergroup the 6 forward layouts are built with
DVE 32x32 block transposes (a <-> innermost free axis) and strided copies
(free-axis swaps); engine assignment follows HW-measured rates (ACT does
strided 2B copies at ~4.2 us/tile vs DVE ~2.1 us, while contiguous
step=-1 reversal copies hit the 2x packed path on DVE at ~1.0 us and run
~2.4 us on ACT):

    DVE: T_ikj=fswap(A), T_kji=transp(A), T_jki=transp(T_ikj),
         T_jik=fswap(T_jki), T_kij=fswap(T_kji), F_kij=rev(T_kij)
    ACT: FA=rev(A), F_ikj=rev(T_ikj), F_kji=rev(T_kji),
         F_jki=rev(T_jki), F_jik=rev(T_jik)

where rev() reverses the free dim within each u block. Each reversed scan
s+2 is then the store of the matching F tile through a DRAM AP whose
a-axis runs backwards (negative outer stride) — the partition half of the
reversal costs nothing. Stores are 512 KB HWDGE DMAs (2 KB runs) on the
qSP ring. Measured ~78.7 us/core for the previous bf16 G-chain variant;
this layout targets the ~41 us store roofline (HW-measured 464 GB/s
effective store rate) plus ramp.
"""

import numpy as np

import concourse.bacc as bacc
import concourse.mybir as mybir
from concourse.tile import TileContext
from concourse.bass_utils import run_bass_kernel_spmd

B = 2
C = 96
D = 32
L = D * D * D            # 32768
NV = B * C               # 192 volumes
NCORES = 8
VPC = NV // NCORES       # 24 volumes per core
SG = 8                   # volumes per supergroup
NSG = VPC // SG          # 3 supergroups per core
F2 = 2 * D * D           # 2048 free elements per partition row

FP32 = mybir.dt.float32
BF16 = mybir.dt.bfloat16

_PROGRAM_CACHE = {}


def _emit(nc, pool, x_in, out):
    for h in range(NSG):
        base = h * SG

        def dram_ap(s):
            # DRAM AP in SBUF stream order: (v, a) partition-major, then
            # (u, f) — element (vol = base+4u+v, a, f) of out[s].
            return (
                out[s, base:base + SG]
                .rearrange("(u v) (a f) -> v a u f", u=2, a=D)
            )

        def tile(tag):
            return pool.tile([128, F2], BF16, tag=tag)

        A = tile("A")
        nc.gpsimd.dma_start(
            out=A[:],
            in_=x_in[base:base + SG].rearrange("(u v) a j k -> v a u j k", u=2),
        )

        def fswap(dst, src):
            # dst[p, u, x, y] = src[p, u, y, x]: swap the two free sub-axes
            nc.vector.tensor_copy(
                out=dst.rearrange("p (u x y) -> p u x y", u=2, x=D),
                in_=src.rearrange("p (u y x) -> p u x y", u=2, y=D),
            )

        def rev(dst, src, eng):
            # dst[p, u, f] = src[p, u, 1023-f]
            eng(
                out=dst.rearrange("p (u f) -> p u f", u=2),
                in_=src.rearrange("p (u f) -> p u f", u=2)[:, :, ::-1],
            )

        # Interleaved so each engine's in-order queue never stalls on the
        # other: every ACT rev input is produced well before ACT reaches it.
        T_ikj = tile("T_ikj")
        fswap(T_ikj, A)                                   # DVE
        FA = tile("FA")
        rev(FA, A, nc.scalar.copy)                        # ACT
        T_kji = tile("T_kji")
        nc.vector.transpose(out=T_kji[:], in_=A[:])       # DVE
        F_ikj = tile("F_ikj")
        rev(F_ikj, T_ikj, nc.scalar.copy)                 # ACT
        T_jki = tile("T_jki")
        nc.vector.transpose(out=T_jki[:], in_=T_ikj[:])   # DVE
        F_kji = tile("F_kji")
        rev(F_kji, T_kji, nc.scalar.copy)                 # ACT
        T_jik = tile("T_jik")
        fswap(T_jik, T_jki)                               # DVE
        F_jki = tile("F_jki")
        rev(F_jki, T_jki, nc.scalar.copy)                 # ACT
        T_kij = tile("T_kij")
        fswap(T_kij, T_kji)                               # DVE
        F_jik = tile("F_jik")
        rev(F_jik, T_jik, nc.scalar.copy)                 # ACT
        F_kij = tile("F_kij")
        rev(F_kij, T_kij, nc.vector.tensor_copy)          # DVE

        def store(s, t, rv=False):
            if not rv:
                nc.sync.dma_start(out=dram_ap(s), in_=t[:])
                return
            # Reversed scan: partition a of the F tile lands at out
            # position 31-a (negative outer stride). The reversed a axis
            # can't merge with f, so stay within the 3-dim DMA AP limit by
            # storing each u half (4 volumes, 256 KB) separately.
            for u in range(2):
                nc.sync.dma_start(
                    out=out[s, base + 4 * u:base + 4 * u + 4]
                    .rearrange("v (a f) -> v a f", a=D)[:, ::-1, :],
                    in_=t.rearrange("p (u f) -> p u f", u=2)[:, u, :],
                )

        # Issue in production order so buffers free promptly.
        store(0, A)
        store(1, T_ikj)
        store(2, FA, True)
        store(9, T_kji)
        store(3, F_ikj, True)
        store(4, T_jki)
        store(11, F_kji, True)
        store(5, T_jik)
        store(6, F_jki, True)
        store(8, T_kij)
        store(7, F_jik, True)
        store(10, F_kij, True)


_TAGS = (
    "A", "FA", "T_ikj", "F_ikj", "T_kji", "F_kji",
    "T_jki", "F_jki", "T_jik", "F_jik", "T_kij", "F_kij",
)


class _Pool:
    """Per-tag tile pools, double-buffered for cross-supergroup pipelining."""

    def __init__(self, tc):
        self.tc = tc
        self.cms = {}
        self.pools = {}

    def __enter__(self):
        return self

    def __exit__(self, *exc):
        for cm in reversed(list(self.cms.values())):
            cm.__exit__(*exc)

    def tile(self, shape, dtype, tag):
        if tag not in self.pools:
            cm = self.tc.tile_pool(name=f"pool_{tag}", bufs=2)
            self.cms[tag] = cm
            self.pools[tag] = cm.__enter__()
        return self.pools[tag].tile(shape, dtype, tag=tag, name=tag)


def build_program(loop_n=None):
    """SPMD program per core: x[VPC, 32, 32, 32] f32 -> out[12, VPC, L] bf16.

    loop_n wraps the workload in a hardware loop re-executing it loop_n
    times (idempotent writes) — used only for performance measurement.
    """
    nc = bacc.Bacc("TRN2", target_bir_lowering=False)
    x_in = nc.dram_tensor("x", [VPC, D, D, D], FP32, kind="ExternalInput")
    out = nc.dram_tensor("out", [12, VPC, L], BF16, kind="ExternalOutput")

    with TileContext(nc) as tc:
        with _Pool(tc) as pool:
            if loop_n:
                with tc.For_i(0, loop_n, 1):
                    _emit(nc, pool, x_in, out)
            else:
                _emit(nc, pool, x_in, out)
    nc.compile()
    return nc


def build_timing_program(loop_n, **kw):
    return build_program(loop_n=loop_n, **kw)


def get_program():
    if "nc" not in _PROGRAM_CACHE:
        _PROGRAM_CACHE["nc"] = build_program()
    return _PROGRAM_CACHE["nc"]


def make_in_maps(x: np.ndarray):
    xf = np.ascontiguousarray(x.astype(np.float32, copy=False)).reshape(NV, D, D, D)
    return [
        {"x": np.ascontiguousarray(xf[m * VPC:(m + 1) * VPC])} for m in range(NCORES)
    ]


def assemble(results) -> np.ndarray:
    out = np.empty((B, 12, C, L), np.float32)
    for m in range(NCORES):
        o = np.asarray(results[m]["out"]).astype(np.float32).reshape(12, VPC, L)
        b, c0 = divmod(m * VPC, C)
        out[b, :, c0:c0 + VPC, :] = o
    return out


def kernel(x: np.ndarray) -> np.ndarray:
    nc = get_program()
    res = run_bass_kernel_spmd(nc, make_in_maps(np.asarray(x)), list(range(NCORES)))
    return assemble(res.results)


# revision 14
# speedup vs baseline: 2.1422x; 2.1422x over previous
"""CrossScan3D Trainium2 kernel.

Computes, for input x[B=2, C=96, 32, 32, 32] f32, the stack of 12 scans
out[B, 12, C, L=32768]: the 6 axis-order flattenings {ijk, ikj, jki, jik,
kij, kji} of each (b, c) 32^3 volume plus their reversals, in the channel
order of the reference:

    s=0: ijk   s=1: ikj   s=2: rev-ijk   s=3: rev-ikj
    s=4: jki   s=5: jik   s=6: rev-jki   s=7: rev-jik
    s=8: kij   s=9: kji   s=10: rev-kij  s=11: rev-kji

Pure data movement; rel-err tolerance is 2e-2, so the DRAM output is bf16
(max rounding rel err ~3.9e-3) and upcast to f32 on the host — this halves
the HBM write traffic that is the roofline. Sharding: the 192 (b, c)
volumes split 24 per core across 8 cores (no communication).

Per core, volumes are processed 8 at a time in [128, 2048] bf16 SBUF
tiles: partition p = v*32 + a (v in 0..3), free = u*1024 + f (u in 0..1),
with volume = base + 4u + v. The load DMA (SWDGE/gpsimd ring) casts
f32->bf16 in flight. All 12 stores are forward-ascending 512 KB HWDGE
DMAs (2 KB runs) — HW-measured at ~464 GB/s effective, while any
reversed-order store AP collapses to < 100 GB/s, so every scan is fully
materialized in SBUF in store order first.

Engine split (from HW-measured op rates: ACT strided 2B copy 4.2 us/tile
vs DVE 2.1 us; DVE shuffle ~2.3 us; ACT PSUM-evac ~2.4 us):

    DVE  (~13.0 us/sg): T_ikj=fswap(A), T_kji=transp(A),
         T_jki=transp(T_ikj), T_jik=fswap(T_jki), T_kij=fswap(T_kji),
         FA=stream_shuffle full reversal of A
    PE   (~7 us/sg, else idle): partition-reverse (block-diag 32x32
         exchange matmul) each of the 5 derived T tiles into PSUM
    ACT  (~11.9 us/sg): evacuate each PSUM tile to SBUF bf16 with the
         free-dim reversal folded into the copy -> full reversal F tiles

which keeps every engine under the ~13.6 us/supergroup store time.
"""

import numpy as np

import concourse.bacc as bacc
import concourse.mybir as mybir
from concourse.tile import TileContext
from concourse.bass_utils import run_bass_kernel_spmd

B = 2
C = 96
D = 32
L = D * D * D            # 32768
NV = B * C               # 192 volumes
NCORES = 8
VPC = NV // NCORES       # 24 volumes per core
SG = 8                   # volumes per supergroup
NSG = VPC // SG          # 3 supergroups per core
F2 = 2 * D * D           # 2048 free elements per partition row

FP32 = mybir.dt.float32
BF16 = mybir.dt.bfloat16

_PROGRAM_CACHE = {}


def _emit(nc, pool, psum_pool, x_in, out, eye_t):
    for h in range(NSG):
        base = h * SG

        def dram_ap(s):
            # DRAM AP in SBUF stream order: (v, a) partition-major, then
            # (u, f) — element (vol = base+4u+v, a, f) of out[s].
            return (
                out[s, base:base + SG]
                .rearrange("(u v) (a f) -> v a u f", u=2, a=D)
            )

        def tile(tag):
            return pool.tile([128, F2], BF16, tag=tag)

        A = tile("A")
        nc.gpsimd.dma_start(
            out=A[:],
            in_=x_in[base:base + SG].rearrange("(u v) a j k -> v a u j k", u=2),
        )

        def fswap(dst, src):
            # dst[p, u, x, y] = src[p, u, y, x]: swap the two free sub-axes
            nc.vector.tensor_copy(
                out=dst.rearrange("p (u x y) -> p u x y", u=2, x=D),
                in_=src.rearrange("p (u y x) -> p u x y", u=2, y=D),
            )

        def fullrev(dst, src):
            # dst = scan-order reversal of src: partition a -> 31-a within
            # each v group (shuffle mask) + free f -> 1023-f within each u
            for u in range(2):
                nc.vector.stream_shuffle(
                    dst.rearrange("p (u f) -> p u f", u=2)[:, u, :],
                    src.rearrange("p (u f) -> p u f", u=2)[:, u, ::-1],
                    list(range(31, -1, -1)),
                )

        def perev_evac(dst, src):
            # Same full reversal, off DVE: PE multiplies by the block-diag
            # 32x32 exchange matrix (partition a -> 31-a, exact 0/1 sums)
            # into PSUM, ACT evacuates with the free reversal + bf16 cast.
            ps = psum_pool.tile([128, F2], FP32, tag="ps")
            for n in range(4):
                nc.tensor.matmul(
                    out=ps[:, 512 * n:512 * (n + 1)],
                    lhsT=eye_t[:],
                    rhs=src[:, 512 * n:512 * (n + 1)],
                    start=True,
                    stop=True,
                )
            nc.scalar.copy(
                out=dst.rearrange("p (u f) -> p u f", u=2),
                in_=ps.rearrange("p (u f) -> p u f", u=2)[:, :, ::-1],
            )

        # DVE queue: forward family + FA. PE/ACT queues: the 5 reversals
        # of the derived tiles, started as soon as each input lands.
        T_ikj = tile("T_ikj")
        fswap(T_ikj, A)                                   # DVE
        F_ikj = tile("F_ikj")
        perev_evac(F_ikj, T_ikj)                          # PE + ACT
        T_kji = tile("T_kji")
        nc.vector.transpose(out=T_kji[:], in_=A[:])       # DVE
        F_kji = tile("F_kji")
        perev_evac(F_kji, T_kji)                          # PE + ACT
        T_jki = tile("T_jki")
        nc.vector.transpose(out=T_jki[:], in_=T_ikj[:])   # DVE
        F_jki = tile("F_jki")
        perev_evac(F_jki, T_jki)                          # PE + ACT
        T_jik = tile("T_jik")
        fswap(T_jik, T_jki)                               # DVE
        F_jik = tile("F_jik")
        perev_evac(F_jik, T_jik)                          # PE + ACT
        T_kij = tile("T_kij")
        fswap(T_kij, T_kji)                               # DVE
        F_kij = tile("F_kij")
        perev_evac(F_kij, T_kij)                          # PE + ACT
        FA = tile("FA")
        fullrev(FA, A)                                    # DVE

        def store(s, t):
            nc.sync.dma_start(out=dram_ap(s), in_=t[:])

        # Issue roughly in production order so buffers free promptly.
        store(0, A)
        store(1, T_ikj)
        store(3, F_ikj)
        store(9, T_kji)
        store(11, F_kji)
        store(4, T_jki)
        store(6, F_jki)
        store(5, T_jik)
        store(7, F_jik)
        store(8, T_kij)
        store(10, F_kij)
        store(2, FA)


_TAGS = (
    "A", "FA", "T_ikj", "F_ikj", "T_kji", "F_kji",
    "T_jki", "F_jki", "T_jik", "F_jik", "T_kij", "F_kij",
)


class _Pool:
    """Per-tag tile pools, double-buffered for cross-supergroup pipelining."""

    def __init__(self, tc):
        self.tc = tc
        self.cms = {}
        self.pools = {}

    def __enter__(self):
        return self

    def __exit__(self, *exc):
        for cm in reversed(list(self.cms.values())):
            cm.__exit__(*exc)

    def tile(self, shape, dtype, tag):
        if tag not in self.pools:
            cm = self.tc.tile_pool(name=f"pool_{tag}", bufs=2)
            self.cms[tag] = cm
            self.pools[tag] = cm.__enter__()
        return self.pools[tag].tile(shape, dtype, tag=tag, name=tag)


def make_eye() -> np.ndarray:
    """Block-diag 32x32 exchange matrix: W[32v + a, 32v + (31-a)] = 1."""
    w = np.zeros((128, 128), np.float32)
    for v in range(4):
        for a in range(D):
            w[32 * v + a, 32 * v + (D - 1 - a)] = 1.0
    return w


def build_program(loop_n=None):
    """SPMD program per core: x[VPC, 32, 32, 32] f32 -> out[12, VPC, L] bf16.

    loop_n wraps the workload in a hardware loop re-executing it loop_n
    times (idempotent writes) — used only for performance measurement.
    """
    nc = bacc.Bacc("TRN2", target_bir_lowering=False)
    x_in = nc.dram_tensor("x", [VPC, D, D, D], FP32, kind="ExternalInput")
    eye_in = nc.dram_tensor("eye", [128, 128], FP32, kind="ExternalInput")
    out = nc.dram_tensor("out", [12, VPC, L], BF16, kind="ExternalOutput")

    with TileContext(nc) as tc:
        with tc.tile_pool(name="eye", bufs=1) as eye_pool, \
             tc.tile_pool(name="psum", bufs=2, space="PSUM") as psum_pool:
            with _Pool(tc) as pool:
                eye_t = eye_pool.tile([128, 128], BF16, tag="eye", name="eye")
                nc.gpsimd.dma_start(out=eye_t[:], in_=eye_in[:])
                if loop_n:
                    with tc.For_i(0, loop_n, 1):
                        _emit(nc, pool, psum_pool, x_in, out, eye_t)
                else:
                    _emit(nc, pool, psum_pool, x_in, out, eye_t)
    nc.compile()
    return nc


def build_timing_program(loop_n, **kw):
    return build_program(loop_n=loop_n, **kw)


def get_program():
    if "nc" not in _PROGRAM_CACHE:
        _PROGRAM_CACHE["nc"] = build_program()
    return _PROGRAM_CACHE["nc"]


def make_in_maps(x: np.ndarray):
    xf = np.ascontiguousarray(x.astype(np.float32, copy=False)).reshape(NV, D, D, D)
    eye = make_eye()
    return [
        {"x": np.ascontiguousarray(xf[m * VPC:(m + 1) * VPC]), "eye": eye}
        for m in range(NCORES)
    ]


def assemble(results) -> np.ndarray:
    out = np.empty((B, 12, C, L), np.float32)
    for m in range(NCORES):
        o = np.asarray(results[m]["out"]).astype(np.float32).reshape(12, VPC, L)
        b, c0 = divmod(m * VPC, C)
        out[b, :, c0:c0 + VPC, :] = o
    return out


def kernel(x: np.ndarray) -> np.ndarray:
    nc = get_program()
    res = run_bass_kernel_spmd(nc, make_in_maps(np.asarray(x)), list(range(NCORES)))
    return assemble(res.results)
